# revision 5
# baseline (speedup 1.0000x reference)
"""Swin shifted-window attention (nn_AttentionSwinInd) on 8 TRN2 cores.

Strategy: data-parallel over the 512 windows (64/core). Host does the
roll + window partition (pure indexing) and transposes each window to
feature-on-partition layout [D=128, L=196]. Device computes, per window:
  Q^T,K^T (head-padded 32-aligned layouts A/B), V (natural, +ones col),
  per-head scores via 4x row-tiled matmuls, exp on ACT,
  PV with ones-column -> unnormalized O^T + per-query denominators,
  reciprocal + K=1 broadcast matmul -> normalize, projection + bias.
Output Y^T [128, 196] per window, host reverses the windowing.
"""

import numpy as np
import ml_dtypes

BF16 = ml_dtypes.bfloat16

N, T, S, D = 2, 16, 3136, 128
H = W = 56
WT, WH, WW = 4, 7, 7
NH, HD = 8, 16
L = WT * WH * WW          # 196
NWIN = 512                # total windows
NCORES = 8
WPC = NWIN // NCORES      # 64 windows per core
KT0, KT1 = 128, 68        # key tiles (128 + 68 = 196)

_cache = {}


def _build_program():
    import concourse.bass as bass
    import concourse.tile as tile
    from concourse import mybir

    f32 = mybir.dt.float32
    bf16 = mybir.dt.bfloat16

    nc = bass.Bass()

    xT = nc.declare_dram_parameter("xT", [128, WPC * L], bf16, isOutput=False)
    wq_a = nc.declare_dram_parameter("wq_a", [128, 128], bf16, isOutput=False)
    wq_b = nc.declare_dram_parameter("wq_b", [128, 128], bf16, isOutput=False)
    wk_a = nc.declare_dram_parameter("wk_a", [128, 128], bf16, isOutput=False)
    wk_b = nc.declare_dram_parameter("wk_b", [128, 128], bf16, isOutput=False)
    wv = nc.declare_dram_parameter("wv", [128, 128], bf16, isOutput=False)
    pw_a = nc.declare_dram_parameter("pw_a", [128, 128], bf16, isOutput=False)
    pw_b = nc.declare_dram_parameter("pw_b", [128, 128], bf16, isOutput=False)
    pb = nc.declare_dram_parameter("pb", [128, 1], f32, isOutput=False)
    yT = nc.declare_dram_parameter("yT", [128, WPC * L], f32, isOutput=True)

    EXP = mybir.ActivationFunctionType.Exp

    with tile.TileContext(nc) as tc:
        with (
            tc.tile_pool(name="consts", bufs=1) as consts,
            tc.tile_pool(name="sb", bufs=2) as sb,
            tc.tile_pool(name="esb", bufs=2) as esb,
            tc.tile_pool(name="pbank", bufs=4, space="PSUM") as pbank,
            tc.tile_pool(name="pst", bufs=1, space="PSUM") as pst,
        ):
            # constants
            wq_a_s = consts.tile([128, 128], bf16, tag="wq_a")
            wq_b_s = consts.tile([128, 128], bf16, tag="wq_b")
            wk_a_s = consts.tile([128, 128], bf16, tag="wk_a")
            wk_b_s = consts.tile([128, 128], bf16, tag="wk_b")
            wv_s = consts.tile([128, 128], bf16, tag="wv")
            pw_a_s = consts.tile([128, 128], bf16, tag="pw_a")
            pw_b_s = consts.tile([128, 128], bf16, tag="pw_b")
            pb_s = consts.tile([128, 1], f32, tag="pb")
            ones_s = consts.tile([128, 17], bf16, tag="ones")
            nc.sync.dma_start(out=wq_a_s, in_=wq_a[:, :])
            nc.sync.dma_start(out=wq_b_s, in_=wq_b[:, :])
            nc.sync.dma_start(out=wk_a_s, in_=wk_a[:, :])
            nc.sync.dma_start(out=wk_b_s, in_=wk_b[:, :])
            nc.sync.dma_start(out=wv_s, in_=wv[:, :])
            nc.sync.dma_start(out=pw_a_s, in_=pw_a[:, :])
            nc.sync.dma_start(out=pw_b_s, in_=pw_b[:, :])
            nc.sync.dma_start(out=pb_s, in_=pb[:, :])
            nc.vector.memset(ones_s, 1.0)

            for w in range(WPC):
                xt = sb.tile([128, L], bf16, tag="xt")
                nc.sync.dma_start(out=xt, in_=xT[:, w * L:(w + 1) * L])

                # --- Q^T, K^T (A/B halves, head h at partitions 32h..32h+15)
                qa_p = pbank.tile([128, L], f32, tag="pb")
                qb_p = pbank.tile([128, L], f32, tag="pb")
                ka_p = pbank.tile([128, L], f32, tag="pb")
                kb_p = pbank.tile([128, L], f32, tag="pb")
                nc.tensor.matmul(qa_p, wq_a_s, xt, start=True, stop=True)
                nc.tensor.matmul(qb_p, wq_b_s, xt, start=True, stop=True)
                nc.tensor.matmul(ka_p, wk_a_s, xt, start=True, stop=True)
                nc.tensor.matmul(kb_p, wk_b_s, xt, start=True, stop=True)
                qa = sb.tile([128, L], bf16, tag="qa")
                qb = sb.tile([128, L], bf16, tag="qb")
                ka = sb.tile([128, L], bf16, tag="ka")
                kb = sb.tile([128, L], bf16, tag="kb")
                nc.vector.tensor_copy(qa, qa_p)
                nc.scalar.copy(qb, qb_p)
                nc.vector.tensor_copy(ka, ka_p)
                nc.scalar.copy(kb, kb_p)

                # --- V natural [tokens, 128], two key tiles, with ones col
                vp0 = pbank.tile([128, 128], f32, tag="pb")
                vp1 = pbank.tile([KT1, 128], f32, tag="pb")
                nc.tensor.matmul(vp0, xt[:, 0:KT0], wv_s, start=True, stop=True)
                nc.tensor.matmul(vp1, xt[:, KT0:L], wv_s, start=True, stop=True)
                va0 = sb.tile([128, 8, 17], bf16, tag="va0")
                va1 = sb.tile([128, 8, 17], bf16, tag="va1")
                nc.vector.memset(va0[:, :, 0:1], 1.0)
                nc.vector.memset(va1[0:KT1, :, 0:1], 1.0)
                nc.vector.tensor_copy(
                    va0[:, :, 1:17], vp0.rearrange("p (h d) -> p h d", h=8))
                nc.scalar.copy(
                    va1[0:KT1, :, 1:17], vp1.rearrange("p (h d) -> p h d", h=8))

                yt_p = pbank.tile([128, L], f32, tag="pb")

                for half, (qh, kh, hoff) in enumerate(
                        ((qa, ka, 0), (qb, kb, 4))):
                    # --- scores: ST[key, query] per head, 4x row-tiled
                    st = pst.tile([128, 4, 512], f32, tag="st")
                    for h in range(4):
                        p0 = 32 * h
                        nc.tensor.matmul(
                            st[:, h, 0:L],
                            kh[p0:p0 + 16, 0:KT0],
                            qh[p0:p0 + 16, :],
                            start=True, stop=True, tile_position=(p0, 0))
                        nc.tensor.matmul(
                            st[0:KT1, h, L:2 * L],
                            kh[p0:p0 + 16, KT0:L],
                            qh[p0:p0 + 16, :],
                            start=True, stop=True, tile_position=(p0, 0))
                    e = esb.tile([128, 4, 2 * L], bf16, tag="e")
                    nc.scalar.activation(e, st[:, :, 0:2 * L], EXP)

                    # --- PV with ones column: row 32h = denom, +1..+16 = O^T
                    ot_p = pbank.tile([128, L], f32, tag="pb")
                    for h in range(4):
                        p0 = 32 * h
                        nc.tensor.matmul(
                            ot_p[p0:p0 + 17, :],
                            va0[:, hoff + h, :],
                            e[0:KT0, h, 0:L],
                            start=True, stop=False, tile_position=(0, p0))
                        nc.tensor.matmul(
                            ot_p[p0:p0 + 17, :],
                            va1[0:KT1, hoff + h, :],
                            e[0:KT1, h, L:2 * L],
                            start=False, stop=True, tile_position=(0, p0))

                    # --- normalize: recip, K=1 broadcast matmul, multiply
                    rec = sb.tile([128, L], bf16, tag="rec")
                    with nc.allow_low_precision(reason="softmax denom recip"):
                        nc.vector.reciprocal(rec, ot_p)
                    b_p = pbank.tile([128, L], f32, tag="pb")
                    for h in range(4):
                        p0 = 32 * h
                        nc.tensor.matmul(
                            b_p[p0:p0 + 17, :],
                            ones_s[p0:p0 + 1, :],
                            rec[p0:p0 + 1, :],
                            start=True, stop=True, tile_position=(p0, p0))
                    bsb = sb.tile([128, L], bf16, tag="bsb")
                    nc.scalar.copy(bsb, b_p)
                    onrm = sb.tile([128, L], bf16, tag="onrm")
                    nc.vector.tensor_mul(onrm, ot_p, bsb)

                    # --- projection accumulate
                    pw_s = pw_a_s if half == 0 else pw_b_s
                    nc.tensor.matmul(yt_p, pw_s, onrm,
                                     start=(half == 0), stop=(half == 1))

                yt_s = sb.tile([128, L], f32, tag="yt_s")
                nc.vector.tensor_scalar_add(yt_s, yt_p, pb_s)
                nc.sync.dma_start(out=yT[:, w * L:(w + 1) * L], in_=yt_s)

    _split_mm_waits(nc, mybir)
    return nc


def _split_mm_waits(nc, mybir):
    """Walrus allows only one sync-wait on a Matmult: move extra waits onto
    PE NoOps inserted just before the matmul (same engine stream, absolute
    sem-ge waits, so waiting earlier is equivalent)."""
    for fn in nc.m.functions:
        for bb in fn.blocks:
            il = bb.instructions
            i = 0
            while i < len(il):
                inst = il[i]
                si = getattr(inst, "sync_info", None)
                if (not isinstance(inst, mybir.InstNoOp) and si is not None
                        and si.on_wait and len(si.on_wait) > 1):
                    waits = list(si.on_wait)
                    for wsel in waits[:-1]:
                        nop = mybir.InstNoOp(
                            name=nc.get_next_instruction_name(),
                            sync_info=mybir.SyncInfo(
                                on_wait=[wsel], on_update=[]),
                            bass_nofuse=True,
                            engine=inst.engine,
                        )
                        il.insert(i, nop)
                        i += 1
                    inst.sync_info = mybir.SyncInfo(
                        on_wait=[waits[-1]], on_update=list(si.on_update))
                i += 1


def _prep_inputs(x, qkv_w, proj_w, proj_b):
    x4 = x.reshape(N, T, H, W, D)
    xr = np.roll(x4, (-WT // 2, -WH // 2, -WW // 2), axis=(1, 2, 3))
    xw = xr.reshape(N, T // WT, WT, H // WH, WH, W // WW, WW, D)
    xw = xw.transpose(0, 1, 3, 5, 2, 4, 6, 7).reshape(NWIN, L, D)

    Wq = qkv_w[0:128] * (HD ** -0.5)
    Wk = qkv_w[128:256]
    Wv = qkv_w[256:384]

    def head_pad_T(Wm):
        # out[di, 32h+j] = Wm[16h+j, di] for 4 heads, rest zero
        out_a = np.zeros((128, 128), np.float32)
        out_b = np.zeros((128, 128), np.float32)
        for h in range(4):
            out_a[:, 32 * h:32 * h + 16] = Wm[16 * h:16 * h + 16].T
            out_b[:, 32 * h:32 * h + 16] = Wm[16 * (h + 4):16 * (h + 4) + 16].T
        return out_a.astype(BF16), out_b.astype(BF16)

    wq_a, wq_b = head_pad_T(Wq)
    wk_a, wk_b = head_pad_T(Wk)
    wv = Wv.T.astype(BF16)

    # proj lhsT: row 32h+1+j of O^T layout corresponds to di = 16h+j
    pw_a = np.zeros((128, 128), np.float32)
    pw_b = np.zeros((128, 128), np.float32)
    for h in range(4):
        pw_a[32 * h + 1:32 * h + 17, :] = proj_w[:, 16 * h:16 * h + 16].T
        pw_b[32 * h + 1:32 * h + 17, :] = \
            proj_w[:, 16 * (h + 4):16 * (h + 4) + 16].T
    pw_a = pw_a.astype(BF16)
    pw_b = pw_b.astype(BF16)
    pb = proj_b.reshape(128, 1).astype(np.float32)

    in_maps = []
    for c in range(NCORES):
        xw_c = xw[c * WPC:(c + 1) * WPC]                  # [64, 196, 128]
        xT_c = np.ascontiguousarray(
            xw_c.transpose(2, 0, 1).reshape(128, WPC * L)).astype(BF16)
        in_maps.append(dict(
            xT=xT_c, wq_a=wq_a, wq_b=wq_b, wk_a=wk_a, wk_b=wk_b,
            wv=wv, pw_a=pw_a, pw_b=pw_b, pb=pb))
    return in_maps


def _gather_output(results):
    yw = np.empty((NWIN, L, D), np.float32)
    for c in range(NCORES):
        yT_c = results[c]["yT"]                            # [128, 64*196]
        yw[c * WPC:(c + 1) * WPC] = \
            yT_c.reshape(128, WPC, L).transpose(1, 2, 0)
    o = yw.reshape(N, T // WT, H // WH, W // WW, WT, WH, WW, D)
    o = o.transpose(0, 1, 4, 2, 5, 3, 6, 7).reshape(N, T, H, W, D)
    o = np.roll(o, (WT // 2, WH // 2, WW // 2), axis=(1, 2, 3))
    return np.ascontiguousarray(o.reshape(N, T, S, D))


def kernel(x, qkv_w, proj_w, proj_b):
    from concourse.bass_utils import run_bass_kernel_spmd

    x = np.asarray(x, np.float32)
    qkv_w = np.asarray(qkv_w, np.float32)
    proj_w = np.asarray(proj_w, np.float32)
    proj_b = np.asarray(proj_b, np.float32)

    if "nc" not in _cache:
        _cache["nc"] = _build_program()
    in_maps = _prep_inputs(x, qkv_w, proj_w, proj_b)
    res = run_bass_kernel_spmd(_cache["nc"], in_maps, list(range(NCORES)))
    return _gather_output(res.results)


# revision 8
# speedup vs baseline: 1.3749x; 1.3749x over previous
"""Swin shifted-window attention (nn_AttentionSwinInd) on 8 TRN2 cores.

Strategy: data-parallel over the 512 windows (64/core). Host does the
roll + window partition (pure indexing) and transposes each window to
feature-on-partition layout [D=128, L=196]. Device computes, per window:
  Q^T,K^T (head-padded 32-aligned layouts A/B), V (natural, +ones col),
  per-head scores via 4x row-tiled matmuls, exp on ACT,
  PV with ones-column -> unnormalized O^T + per-query denominators,
  reciprocal + K=1 broadcast matmul -> normalize, projection + bias.
Output Y^T [128, 196] per window, host reverses the windowing.
"""

import numpy as np
import ml_dtypes

BF16 = ml_dtypes.bfloat16

N, T, S, D = 2, 16, 3136, 128
H = W = 56
WT, WH, WW = 4, 7, 7
NH, HD = 8, 16
L = WT * WH * WW          # 196
NWIN = 512                # total windows
NCORES = 8
WPC = NWIN // NCORES      # 64 windows per core
KT0, KT1 = 128, 68        # key tiles (128 + 68 = 196)

_cache = {}


def _build_program():
    import concourse.bass as bass
    import concourse.tile as tile
    from concourse import mybir

    f32 = mybir.dt.float32
    bf16 = mybir.dt.bfloat16

    nc = bass.Bass()

    xT = nc.declare_dram_parameter("xT", [128, WPC * L], bf16, isOutput=False)
    wq_a = nc.declare_dram_parameter("wq_a", [128, 128], bf16, isOutput=False)
    wq_b = nc.declare_dram_parameter("wq_b", [128, 128], bf16, isOutput=False)
    wk_a = nc.declare_dram_parameter("wk_a", [128, 128], bf16, isOutput=False)
    wk_b = nc.declare_dram_parameter("wk_b", [128, 128], bf16, isOutput=False)
    wv = nc.declare_dram_parameter("wv", [128, 128], bf16, isOutput=False)
    pw_a = nc.declare_dram_parameter("pw_a", [128, 128], bf16, isOutput=False)
    pw_b = nc.declare_dram_parameter("pw_b", [128, 128], bf16, isOutput=False)
    pb = nc.declare_dram_parameter("pb", [128, 1], f32, isOutput=False)
    yT = nc.declare_dram_parameter("yT", [128, WPC * L], f32, isOutput=True)

    EXP = mybir.ActivationFunctionType.Exp

    with tile.TileContext(nc) as tc:
        with (
            tc.tile_pool(name="consts", bufs=1) as consts,
            tc.tile_pool(name="sb", bufs=2) as sb,
            tc.tile_pool(name="esb", bufs=2) as esb,
            tc.tile_pool(name="pbank", bufs=4, space="PSUM") as pbank,
            tc.tile_pool(name="pst", bufs=1, space="PSUM") as pst,
        ):
            # constants
            wq_a_s = consts.tile([128, 128], bf16, tag="wq_a")
            wq_b_s = consts.tile([128, 128], bf16, tag="wq_b")
            wk_a_s = consts.tile([128, 128], bf16, tag="wk_a")
            wk_b_s = consts.tile([128, 128], bf16, tag="wk_b")
            wv_s = consts.tile([128, 128], bf16, tag="wv")
            pw_a_s = consts.tile([128, 128], bf16, tag="pw_a")
            pw_b_s = consts.tile([128, 128], bf16, tag="pw_b")
            pb_s = consts.tile([128, 1], f32, tag="pb")
            ones_s = consts.tile([128, 17], bf16, tag="ones")
            nc.sync.dma_start(out=wq_a_s, in_=wq_a[:, :])
            nc.sync.dma_start(out=wq_b_s, in_=wq_b[:, :])
            nc.sync.dma_start(out=wk_a_s, in_=wk_a[:, :])
            nc.sync.dma_start(out=wk_b_s, in_=wk_b[:, :])
            nc.sync.dma_start(out=wv_s, in_=wv[:, :])
            nc.sync.dma_start(out=pw_a_s, in_=pw_a[:, :])
            nc.sync.dma_start(out=pw_b_s, in_=pw_b[:, :])
            nc.sync.dma_start(out=pb_s, in_=pb[:, :])
            nc.vector.memset(ones_s, 1.0)

            for w in range(WPC):
                xt = sb.tile([128, L], bf16, tag="xt")
                nc.sync.dma_start(out=xt, in_=xT[:, w * L:(w + 1) * L])

                # --- Q^T, K^T (A/B halves, head h at partitions 32h..32h+15)
                qa_p = pbank.tile([128, L], f32, tag="pb")
                qb_p = pbank.tile([128, L], f32, tag="pb")
                ka_p = pbank.tile([128, L], f32, tag="pb")
                kb_p = pbank.tile([128, L], f32, tag="pb")
                nc.tensor.matmul(qa_p, wq_a_s, xt, start=True, stop=True)
                nc.tensor.matmul(qb_p, wq_b_s, xt, start=True, stop=True)
                nc.tensor.matmul(ka_p, wk_a_s, xt, start=True, stop=True)
                nc.tensor.matmul(kb_p, wk_b_s, xt, start=True, stop=True)
                qa = sb.tile([128, L], bf16, tag="qa")
                qb = sb.tile([128, L], bf16, tag="qb")
                ka = sb.tile([128, L], bf16, tag="ka")
                kb = sb.tile([128, L], bf16, tag="kb")
                nc.vector.tensor_copy(qa, qa_p)
                nc.vector.tensor_copy(qb, qb_p)
                nc.vector.tensor_copy(ka, ka_p)
                nc.vector.tensor_copy(kb, kb_p)

                # --- V natural [tokens, 128], two key tiles, with ones col
                vp0 = pbank.tile([128, 128], f32, tag="pb")
                vp1 = pbank.tile([KT1, 128], f32, tag="pb")
                nc.tensor.matmul(vp0, xt[:, 0:KT0], wv_s, start=True, stop=True)
                nc.tensor.matmul(vp1, xt[:, KT0:L], wv_s, start=True, stop=True)
                va0 = sb.tile([128, 8, 17], bf16, tag="va0")
                va1 = sb.tile([128, 8, 17], bf16, tag="va1")
                nc.vector.memset(va0[:, :, 0:1], 1.0)
                nc.vector.memset(va1[0:KT1, :, 0:1], 1.0)
                nc.vector.tensor_copy(
                    va0[:, :, 1:17], vp0.rearrange("p (h d) -> p h d", h=8))
                nc.vector.tensor_copy(
                    va1[0:KT1, :, 1:17], vp1.rearrange("p (h d) -> p h d", h=8))

                yt_p = pbank.tile([128, L], f32, tag="pb")

                for half, (qh, kh, hoff) in enumerate(
                        ((qa, ka, 0), (qb, kb, 4))):
                    # --- scores: ST[key, query] per head, 4x row-tiled
                    st = pst.tile([128, 4, 512], f32, tag="st")
                    for h in range(4):
                        p0 = 32 * h
                        nc.tensor.matmul(
                            st[:, h, 0:L],
                            kh[p0:p0 + 16, 0:KT0],
                            qh[p0:p0 + 16, :],
                            start=True, stop=True, tile_position=(p0, 0))
                        nc.tensor.matmul(
                            st[0:KT1, h, L:2 * L],
                            kh[p0:p0 + 16, KT0:L],
                            qh[p0:p0 + 16, :],
                            start=True, stop=True, tile_position=(p0, 0))
                    e = esb.tile([128, 4, 2 * L], bf16, tag="e")
                    nc.scalar.activation(e, st[:, :, 0:2 * L], EXP)

                    # --- PV with ones column: row 32h = denom, +1..+16 = O^T
                    ot_p = pbank.tile([128, L], f32, tag="pb")
                    for h in range(4):
                        p0 = 32 * h
                        nc.tensor.matmul(
                            ot_p[p0:p0 + 17, :],
                            va0[:, hoff + h, :],
                            e[0:KT0, h, 0:L],
                            start=True, stop=False, tile_position=(0, p0))
                        nc.tensor.matmul(
                            ot_p[p0:p0 + 17, :],
                            va1[0:KT1, hoff + h, :],
                            e[0:KT1, h, L:2 * L],
                            start=False, stop=True, tile_position=(0, p0))

                    # --- normalize: recip, K=1 broadcast matmul, multiply
                    rec = sb.tile([128, L], bf16, tag="rec")
                    with nc.allow_low_precision(reason="softmax denom recip"):
                        nc.vector.reciprocal(rec, ot_p)
                    b_p = pbank.tile([128, L], f32, tag="pb")
                    for h in range(4):
                        p0 = 32 * h
                        nc.tensor.matmul(
                            b_p[p0:p0 + 17, :],
                            ones_s[p0:p0 + 1, :],
                            rec[p0:p0 + 1, :],
                            start=True, stop=True, tile_position=(p0, p0))
                    bsb = sb.tile([128, L], bf16, tag="bsb")
                    nc.scalar.copy(bsb, b_p)
                    onrm = sb.tile([128, L], bf16, tag="onrm")
                    nc.vector.tensor_mul(onrm, ot_p, bsb)

                    # --- projection accumulate
                    pw_s = pw_a_s if half == 0 else pw_b_s
                    nc.tensor.matmul(yt_p, pw_s, onrm,
                                     start=(half == 0), stop=(half == 1))

                yt_s = sb.tile([128, L], f32, tag="yt_s")
                nc.vector.tensor_scalar_add(yt_s, yt_p, pb_s)
                nc.sync.dma_start(out=yT[:, w * L:(w + 1) * L], in_=yt_s)

    _split_mm_waits(nc, mybir)
    return nc


def _split_mm_waits(nc, mybir):
    """Walrus allows only one sync-wait on a Matmult: move extra waits onto
    PE NoOps inserted just before the matmul (same engine stream, absolute
    sem-ge waits, so waiting earlier is equivalent)."""
    for fn in nc.m.functions:
        for bb in fn.blocks:
            il = bb.instructions
            i = 0
            while i < len(il):
                inst = il[i]
                si = getattr(inst, "sync_info", None)
                if (not isinstance(inst, mybir.InstNoOp) and si is not None
                        and si.on_wait and len(si.on_wait) > 1):
                    waits = list(si.on_wait)
                    for wsel in waits[:-1]:
                        nop = mybir.InstNoOp(
                            name=nc.get_next_instruction_name(),
                            sync_info=mybir.SyncInfo(
                                on_wait=[wsel], on_update=[]),
                            bass_nofuse=True,
                            engine=inst.engine,
                        )
                        il.insert(i, nop)
                        i += 1
                    inst.sync_info = mybir.SyncInfo(
                        on_wait=[waits[-1]], on_update=list(si.on_update))
                i += 1


def _prep_inputs(x, qkv_w, proj_w, proj_b):
    x4 = x.reshape(N, T, H, W, D)
    xr = np.roll(x4, (-WT // 2, -WH // 2, -WW // 2), axis=(1, 2, 3))
    xw = xr.reshape(N, T // WT, WT, H // WH, WH, W // WW, WW, D)
    xw = xw.transpose(0, 1, 3, 5, 2, 4, 6, 7).reshape(NWIN, L, D)

    Wq = qkv_w[0:128] * (HD ** -0.5)
    Wk = qkv_w[128:256]
    Wv = qkv_w[256:384]

    def head_pad_T(Wm):
        # out[di, 32h+j] = Wm[16h+j, di] for 4 heads, rest zero
        out_a = np.zeros((128, 128), np.float32)
        out_b = np.zeros((128, 128), np.float32)
        for h in range(4):
            out_a[:, 32 * h:32 * h + 16] = Wm[16 * h:16 * h + 16].T
            out_b[:, 32 * h:32 * h + 16] = Wm[16 * (h + 4):16 * (h + 4) + 16].T
        return out_a.astype(BF16), out_b.astype(BF16)

    wq_a, wq_b = head_pad_T(Wq)
    wk_a, wk_b = head_pad_T(Wk)
    wv = Wv.T.astype(BF16)

    # proj lhsT: row 32h+1+j of O^T layout corresponds to di = 16h+j
    pw_a = np.zeros((128, 128), np.float32)
    pw_b = np.zeros((128, 128), np.float32)
    for h in range(4):
        pw_a[32 * h + 1:32 * h + 17, :] = proj_w[:, 16 * h:16 * h + 16].T
        pw_b[32 * h + 1:32 * h + 17, :] = \
            proj_w[:, 16 * (h + 4):16 * (h + 4) + 16].T
    pw_a = pw_a.astype(BF16)
    pw_b = pw_b.astype(BF16)
    pb = proj_b.reshape(128, 1).astype(np.float32)

    in_maps = []
    for c in range(NCORES):
        xw_c = xw[c * WPC:(c + 1) * WPC]                  # [64, 196, 128]
        xT_c = np.ascontiguousarray(
            xw_c.transpose(2, 0, 1).reshape(128, WPC * L)).astype(BF16)
        in_maps.append(dict(
            xT=xT_c, wq_a=wq_a, wq_b=wq_b, wk_a=wk_a, wk_b=wk_b,
            wv=wv, pw_a=pw_a, pw_b=pw_b, pb=pb))
    return in_maps


def _gather_output(results):
    yw = np.empty((NWIN, L, D), np.float32)
    for c in range(NCORES):
        yT_c = results[c]["yT"]                            # [128, 64*196]
        yw[c * WPC:(c + 1) * WPC] = \
            yT_c.reshape(128, WPC, L).transpose(1, 2, 0)
    o = yw.reshape(N, T // WT, H // WH, W // WW, WT, WH, WW, D)
    o = o.transpose(0, 1, 4, 2, 5, 3, 6, 7).reshape(N, T, H, W, D)
    o = np.roll(o, (WT // 2, WH // 2, WW // 2), axis=(1, 2, 3))
    return np.ascontiguousarray(o.reshape(N, T, S, D))


def kernel(x, qkv_w, proj_w, proj_b):
    from concourse.bass_utils import run_bass_kernel_spmd

    x = np.asarray(x, np.float32)
    qkv_w = np.asarray(qkv_w, np.float32)
    proj_w = np.asarray(proj_w, np.float32)
    proj_b = np.asarray(proj_b, np.float32)

    if "nc" not in _cache:
        _cache["nc"] = _build_program()
    in_maps = _prep_inputs(x, qkv_w, proj_w, proj_b)
    import os
    trace = bool(os.environ.get("SWIN_TRACE"))
    res = run_bass_kernel_spmd(_cache["nc"], in_maps, list(range(NCORES)),
                               trace=trace)
    if trace:
        _cache["last_exec_time_ns"] = res.exec_time_ns
        _cache["last_profile"] = res.profile_json
    return _gather_output(res.results)
